# revision 19
# baseline (speedup 1.0000x reference)
"""Trainium2 Bass kernel for nn_CAM (DANet channel-attention module).

Per batch element b (one per NeuronCore, 8 cores data-parallel over B=8):
    xf = x[b].reshape(C, H*W)                       # [512, 4096]
    E = xf @ xf.T                                   # [512, 512] (symmetric)
    att = softmax(max_j(E) - E, axis=-1)            # inverted softmax
    out = gamma * (att @ xf) + x[b]

Kernel math (identical in exact arithmetic to the reference):
    c[i]    = min_j E[i, j]         (= column min by symmetry)
    W[j, i] = exp(c[i] - E[j, i])   (= numerator of att[i, j]; exponent <= 0)
    S[i]    = sum_j W[j, i]
    out[i]  = gamma * (1/S[i]) * sum_j W[j, i] * xf[j, :] + x[b][i, :]

Design notes (v3 — software-pipelined across reps):
  - fp16 matmul path: same 10-bit mantissa as tf32/f32r (end-to-end rel err
    ~1e-2 at gamma=1) but full PE rate for matmuls AND transposes. PSUM
    accumulation is fp32; the residual is added from the fp16 copy of x
    (adds ~5e-4 rel err, far under the 2e-2 gate).
  - per-rep work is split A (load->cast->transpose->mm1), B1 (symmetrize,
    rowmin, W=exp(c-E)) and B2 (S, mm2, epilogue, store). Emission is
    SKEWED: A(r), B1(r), B2(r-1) — so the PE rolls from rep r's mm1
    straight into rep r-1's mm2 while rep r's softmax chain (Pool/ACT)
    runs concurrently. The PE p-state ramps to full clock only after
    ~3us without stalls, so gaps are doubly expensive.
  - no full-size fp32 copy of x in SBUF: loads staged in 512-col chunks,
    cast fp32->fp16 on the Pool engine into a triple-buffered fp16 Xh
    (3 live reps: consumed by B2(r-1), ready for B2(r), written by A(r+1)).
  - B1's elementwise work (rowmin, c_sb, subtract) on Pool so the DVE
    queue holds only mm2-epilogue work when mm2 starts.
  - loads exclusively on the SP queue, stores on the ACT queue; one 1MB
    store per chunk (fewer HWDGE descriptor-generation serializations).
  - epilogue: out = invsg_i * (W^T @ xf) + xh in one DVE op per block;
    mm2 never waits on the softmax denominator.
  - reps chain through DRAM with lag 2 (rep r reads rep r-2's output,
    3 buffer sets) to match the emission skew; the final rep still
    writes the real fp32 output.

reps > 1 unrolls the whole computation serially inside one NEFF with a
true data chain between reps (used by test.py to measure steady-state
per-iteration device time).
"""

import numpy as np

import concourse.bass as bass
import concourse.mybir as mybir
import concourse.tile as tile
from concourse import bacc
from concourse.masks import make_identity

P = 128          # partitions
C = 512          # channels
HW = 4096        # spatial (64*64)
CB = C // P      # 4 channel blocks
KB = HW // P     # 32 spatial blocks
NW = 512         # matmul free-dim chunk
NCH = HW // NW   # 8 n-chunks

F32 = mybir.dt.float32
F16 = mybir.dt.float16
EXP = mybir.ActivationFunctionType.Exp
ALU = mybir.AluOpType
AX = mybir.AxisListType

# symmetry: matmuls compute only blocks (jb, ib) with ib >= jb
RS = (0, P, 2 * P, 3 * P)
SYM = ((1, 0), (2, 0), (2, 1), (3, 0), (3, 1), (3, 2))
# batched copies of the upper-triangle blocks feeding the SYM transposes:
# (src_jb, src_col_block_start, n_blocks)
SYMCP = ((0, 1, 3), (1, 2, 2), (2, 3, 1))


def build_nc(reps: int = 1):
    nc = bacc.Bacc("TRN2", target_bir_lowering=False)
    x = nc.dram_tensor("x", [C, HW], F32, kind="ExternalInput")
    g = nc.dram_tensor("gamma", [1], F32, kind="ExternalInput")
    y = nc.dram_tensor("y", [C, HW], F32, kind="ExternalOutput")

    with tile.TileContext(nc) as tc:
        with (
            tc.tile_pool(name="stage", bufs=4) as stage,
            tc.tile_pool(name="xh", bufs=3) as xh_pool,
            tc.tile_pool(name="xtr", bufs=4) as xtr_pool,
            tc.tile_pool(name="w", bufs=2) as w_pool,
            tc.tile_pool(name="small", bufs=2) as small,
            tc.tile_pool(name="const", bufs=1) as cst,
            tc.tile_pool(name="esb", bufs=1) as esb_pool,
            tc.tile_pool(name="outp", bufs=4) as outp,
            tc.tile_pool(name="dram", bufs=4, space="DRAM") as dramp,
            tc.tile_pool(name="acc", bufs=4, space="PSUM") as acc_pool,
            tc.tile_pool(name="ptr", bufs=2, space="PSUM") as ptr_pool,
            tc.tile_pool(name="pout", bufs=2, space="PSUM") as pout_pool,
        ):
            # ---- constants (hoisted out of the rep loop)
            ident_f = cst.tile([P, P], F32)
            make_identity(nc, ident_f)
            ident_h = cst.tile([P, P], F16)
            nc.scalar.copy(out=ident_h, in_=ident_f)
            ones_h = cst.tile([P, 8], F16)
            nc.vector.memset(ones_h, 1.0)
            gamma_bc = cst.tile([P, 1], F32)
            nc.gpsimd.dma_start(out=gamma_bc, in_=g[:].partition_broadcast(P))
            # preload the Exp table so the first real exp isn't stalled
            warm = cst.tile([P, 1], F32)
            nc.scalar.activation(out=warm, in_=gamma_bc, func=EXP)

            xr = x.rearrange("(t p) n -> p t n", p=P)
            yr = y.rearrange("(t p) n -> p t n", p=P)

            # lag-3 DRAM chain for reps > 1 (4 buffer sets, NW-wide column
            # tiles): rep r stores to set r%4, rep r reads set (r-3)%4, so
            # rep r+1's loads can prefetch a whole pipeline segment early.
            if reps > 1:
                ybrs = [
                    [
                        dramp.tile(
                            [C, NW], F32, tag=f"ybuf{i}_{ci}",
                            name=f"ybuf{i}_{ci}",
                        ).rearrange("(t p) n -> p t n", p=P)
                        for ci in range(NCH)
                    ]
                    for i in range(4)
                ]

            # ============ phase A: load -> cast -> transpose -> mm1
            def phase_a(rep):
                in_chunks = None if rep < 3 else ybrs[(rep - 3) % 4]

                Xh = xh_pool.tile([P, CB, HW], F16, tag="xh")
                pe_tiles = [
                    acc_pool.tile([P, C], F32, tag="acc", name=f"pe_{_jb}")
                    for _jb in range(CB)
                ]

                def _mm1(k, xt_k):
                    for jb in range(CB):
                        nc.tensor.matmul(
                            pe_tiles[jb][:, RS[jb]:],
                            lhsT=xt_k[:, jb * P:(jb + 1) * P],
                            rhs=xt_k[:, RS[jb]:],
                            start=(k == 0),
                            stop=(k == KB - 1),
                        )

                pend = []
                for ci in range(NCH):
                    pos = ci * NW
                    nsl = slice(pos, pos + NW)
                    src = xr[:, :, nsl] if in_chunks is None else in_chunks[ci]
                    Xs = stage.tile([P, CB, NW], F32, tag="xs")
                    nc.sync.dma_start(out=Xs, in_=src)
                    nc.gpsimd.tensor_copy(out=Xh[:, :, nsl], in_=Xs)
                    for kk in range(NW // P):
                        k = pos // P + kk
                        pxt = ptr_pool.tile([P, C], F16, tag="ptr", name="pxt")
                        for t in range(CB):
                            nc.tensor.transpose(
                                pxt[:, t * P:(t + 1) * P],
                                Xh[:, t, k * P:(k + 1) * P],
                                ident_h,
                            )
                        xt_k = xtr_pool.tile([P, C], F16, tag="xt")
                        nc.vector.tensor_copy(out=xt_k, in_=pxt)
                        # software-pipelined emission: this k's matmuls are
                        # issued one step later so the PE transposes the next
                        # block instead of stalling on the PSUM->SBUF copy
                        pend.append((k, xt_k))
                        if len(pend) > 2:
                            _mm1(*pend.pop(0))

                while pend:
                    _mm1(*pend.pop(0))
                return Xh, pe_tiles

            # ============ phase B1 head: symmetrize via SBUF + column-max
            # (GPSIMD cannot touch PSUM, so E moves to SBUF first on ACT;
            #  that also releases the PSUM accumulators earlier.)
            def phase_b1_head(pe_tiles):
                Esb = esb_pool.tile([P, CB, C], F32, tag="esb")
                cpart = esb_pool.tile([1, CB, C], F32, tag="cpart")
                cmin = esb_pool.tile([1, C], F32, tag="cmin")
                c_sb = esb_pool.tile([P, C], F32, tag="csb")
                W = w_pool.tile([P, CB, C], F16, tag="w")
                Wt = w_pool.tile([P, CB, C], F16, tag="wt")

                # computed (upper-triangle) part of each row-block -> SBUF,
                # negated: Esb holds -E so the cross-lane reduce can use max
                # (min is not supported cross-lane)
                for jb in range(CB):
                    nc.scalar.mul(
                        out=Esb[:, jb, RS[jb]:], in_=pe_tiles[jb][:, RS[jb]:],
                        mul=-1.0,
                    )
                # PE transposes fill the lower blocks (into PSUM)...
                for (bi, bj) in SYM:
                    nc.tensor.transpose(
                        pe_tiles[bi][:, bj * P:(bj + 1) * P],
                        Esb[:, bj, bi * P:(bi + 1) * P], ident_f
                    )
                # ...which then land in SBUF too (batched per row-block)
                for bi in (1, 2, 3):
                    nc.scalar.copy(
                        out=Esb[:, bi, :bi * P], in_=pe_tiles[bi][:, :bi * P]
                    )

                # c is the row min of E; by symmetry also the column min,
                # i.e. -c = column max of -E: the Pool engine reduces along
                # partitions directly into free-axis layout (one partial per
                # row-block; the max-tree runs later on DVE, where there is
                # slack mid-mm2).
                nc.gpsimd.tensor_reduce(
                    out=cpart, in_=Esb, axis=AX.C, op=ALU.max,
                )
                return Esb, cpart, cmin, c_sb, W, Wt

            # ============ phase B1 tail: -c combine, broadcast, W = exp(c-E)
            def phase_b1_tail(ctx):
                Esb, cpart, cmin, c_sb, W, Wt = ctx
                nc.vector.tensor_tensor(
                    out=cpart[:, 0, :], in0=cpart[:, 0, :],
                    in1=cpart[:, 1, :], op=ALU.max,
                )
                nc.vector.tensor_tensor(
                    out=cpart[:, 2, :], in0=cpart[:, 2, :],
                    in1=cpart[:, 3, :], op=ALU.max,
                )
                nc.vector.tensor_tensor(
                    out=cmin, in0=cpart[:, 0, :],
                    in1=cpart[:, 2, :], op=ALU.max,
                )
                # replicate -c down all 128 partitions (small SWDGE copy)
                nc.gpsimd.partition_broadcast(out_ap=c_sb, in_ap=cmin)

                # W = exp(c - E) = (-E) - (-c)  ([j_part, i_free], fp16)
                for jb in range(CB):
                    nc.gpsimd.tensor_tensor(
                        out=Wt[:, jb, :], in0=Esb[:, jb, :], in1=c_sb,
                        op=ALU.subtract,
                    )
                    nc.scalar.activation(
                        out=W[:, jb, :], in_=Wt[:, jb, :], func=EXP
                    )
                return W

            # ============ phase B2s: S sums + invsg (tiny PE work that
            # fills the gap between mm1's end and the symmetrize copies)
            def phase_b2s(W):
                invsg = small.tile([P, CB], F32, tag="invsg")
                S_ps = ptr_pool.tile([P, CB, 8], F32, tag="ptr", name="sps")
                for ib in range(CB):
                    for jb in range(CB):
                        nc.tensor.matmul(
                            S_ps[:, ib, :],
                            lhsT=W[:, jb, ib * P:(ib + 1) * P],
                            rhs=ones_h,
                            start=(jb == 0),
                            stop=(jb == CB - 1),
                        )
                nc.vector.reciprocal(out=invsg, in_=S_ps[:, :, 0])
                for ib in range(CB):
                    nc.vector.tensor_tensor(
                        out=invsg[:, ib:ib + 1], in0=invsg[:, ib:ib + 1],
                        in1=gamma_bc, op=ALU.mult,
                    )
                return invsg

            # ============ phase B2m: mm2 + epilogue + store
            def phase_b2m(rep, Xh, W, invsg, mid_hook=None):
                out_chunks = None if rep == reps - 1 else ybrs[rep % 4]

                # out = invsg_i * (W^T @ xf) + xh, chn-outer; one store/chunk
                for ci in range(NCH):
                    pos = ci * NW
                    nsl = slice(pos, pos + NW)
                    dst = yr[:, :, nsl] if out_chunks is None else out_chunks[ci]
                    osb = outp.tile([P, CB, NW], F32, tag="osb")
                    for ib in range(CB):
                        po_t = pout_pool.tile([P, NW], F32, tag="po", name="po")
                        for jb in range(CB):
                            nc.tensor.matmul(
                                po_t,
                                lhsT=W[:, jb, ib * P:(ib + 1) * P],
                                rhs=Xh[:, jb, nsl],
                                start=(jb == 0),
                                stop=(jb == CB - 1),
                            )
                        nc.vector.scalar_tensor_tensor(
                            out=osb[:, ib, :],
                            in0=po_t,
                            scalar=invsg[:, ib:ib + 1],
                            in1=Xh[:, ib, nsl],
                            op0=ALU.mult,
                            op1=ALU.add,
                        )
                    nc.scalar.dma_start(out=dst, in_=osb)
                    if ci == 3 and mid_hook is not None:
                        mid_hook()

            # ---- skewed emission: A(r), B2s(r-1), B1h(r), B2m(r-1) with
            # B1-tail(r) emitted mid-mm2 where the DVE queue has slack
            prev = None
            for _rep in range(reps):
                Xh_r, pe_r = phase_a(_rep)
                if prev is not None:
                    inv_p = phase_b2s(prev[1])
                ctx_r = phase_b1_head(pe_r)
                if prev is not None:
                    phase_b2m(
                        _rep - 1, prev[0], prev[1], inv_p,
                        mid_hook=lambda c=ctx_r: phase_b1_tail(c),
                    )
                else:
                    phase_b1_tail(ctx_r)
                prev = (Xh_r, ctx_r[4])
            inv_l = phase_b2s(prev[1])
            phase_b2m(reps - 1, prev[0], prev[1], inv_l)

    nc.compile()
    return nc


_NC_CACHE = None


def _get_nc():
    global _NC_CACHE
    if _NC_CACHE is None:
        _NC_CACHE = build_nc()
    return _NC_CACHE


def kernel(x, gamma):
    from concourse.bass_utils import run_bass_kernel_spmd

    x = np.ascontiguousarray(np.asarray(x, dtype=np.float32))
    B = x.shape[0]
    assert x.shape == (8, C, 64, 64), x.shape
    xf = x.reshape(B, C, HW)
    gamma = np.ascontiguousarray(np.asarray(gamma, dtype=np.float32)).reshape(1)

    nc = _get_nc()
    in_maps = [{"x": xf[b], "gamma": gamma} for b in range(B)]
    res = run_bass_kernel_spmd(nc, in_maps, core_ids=list(range(B)))
    out = np.stack([res.results[b]["y"] for b in range(B)], axis=0)
    return out.reshape(B, C, 64, 64).astype(np.float32)


# revision 21
# speedup vs baseline: 3.6208x; 3.6208x over previous
"""Trainium2 Bass kernel for nn_CAM (DANet channel-attention module).

Per batch element b (one per NeuronCore, 8 cores data-parallel over B=8):
    xf = x[b].reshape(C, H*W)                       # [512, 4096]
    E = xf @ xf.T                                   # [512, 512] (symmetric)
    att = softmax(max_j(E) - E, axis=-1)            # inverted softmax
    out = gamma * (att @ xf) + x[b]

Kernel math (identical in exact arithmetic to the reference):
    c[i]    = min_j E[i, j]         (= column min by symmetry)
    W[j, i] = exp(c[i] - E[j, i])   (= numerator of att[i, j]; exponent <= 0)
    S[i]    = sum_j W[j, i]
    out[i]  = gamma * (1/S[i]) * sum_j W[j, i] * xf[j, :] + x[b][i, :]

Design notes (v3 — software-pipelined across reps):
  - fp16 matmul path: same 10-bit mantissa as tf32/f32r (end-to-end rel err
    ~1e-2 at gamma=1) but full PE rate for matmuls AND transposes. PSUM
    accumulation is fp32; the residual is added from the fp16 copy of x
    (adds ~5e-4 rel err, far under the 2e-2 gate).
  - per-rep work is split A (load->cast->transpose->mm1), B1 (symmetrize,
    rowmin, W=exp(c-E)) and B2 (S, mm2, epilogue, store). Emission is
    SKEWED: A(r), B1(r), B2(r-1) — so the PE rolls from rep r's mm1
    straight into rep r-1's mm2 while rep r's softmax chain (Pool/ACT)
    runs concurrently. The PE p-state ramps to full clock only after
    ~3us without stalls, so gaps are doubly expensive.
  - no full-size fp32 copy of x in SBUF: loads staged in 512-col chunks,
    cast fp32->fp16 on the Pool engine into a triple-buffered fp16 Xh
    (3 live reps: consumed by B2(r-1), ready for B2(r), written by A(r+1)).
  - B1's elementwise work (rowmin, c_sb, subtract) on Pool so the DVE
    queue holds only mm2-epilogue work when mm2 starts.
  - loads exclusively on the SP queue, stores on the ACT queue; one 1MB
    store per chunk (fewer HWDGE descriptor-generation serializations).
  - epilogue: out = invsg_i * (W^T @ xf) + xh in one DVE op per block;
    mm2 never waits on the softmax denominator.
  - reps chain through DRAM with lag 2 (rep r reads rep r-2's output,
    3 buffer sets) to match the emission skew; the final rep still
    writes the real fp32 output.

reps > 1 unrolls the whole computation serially inside one NEFF with a
true data chain between reps (used by test.py to measure steady-state
per-iteration device time).
"""

import numpy as np

import concourse.bass as bass
import concourse.mybir as mybir
import concourse.tile as tile
from concourse import bacc
from concourse.masks import make_identity

P = 128          # partitions
C = 512          # channels
HW = 4096        # spatial (64*64)
CB = C // P      # 4 channel blocks
KB = HW // P     # 32 spatial blocks
NW = 512         # matmul free-dim chunk
NCH = HW // NW   # 8 n-chunks

F32 = mybir.dt.float32
F16 = mybir.dt.float16
EXP = mybir.ActivationFunctionType.Exp
ALU = mybir.AluOpType
AX = mybir.AxisListType

# symmetry: matmuls compute only blocks (jb, ib) with ib >= jb
RS = (0, P, 2 * P, 3 * P)
SYM = ((1, 0), (2, 0), (2, 1), (3, 0), (3, 1), (3, 2))
# batched copies of the upper-triangle blocks feeding the SYM transposes:
# (src_jb, src_col_block_start, n_blocks)
SYMCP = ((0, 1, 3), (1, 2, 2), (2, 3, 1))


def build_nc(reps: int = 1):
    nc = bacc.Bacc("TRN2", target_bir_lowering=False)
    x = nc.dram_tensor("x", [C, HW], F32, kind="ExternalInput")
    g = nc.dram_tensor("gamma", [1], F32, kind="ExternalInput")
    y = nc.dram_tensor("y", [C, HW], F32, kind="ExternalOutput")

    with tile.TileContext(nc) as tc:
        with (
            tc.tile_pool(name="stage", bufs=5) as stage,
            tc.tile_pool(name="xh", bufs=3) as xh_pool,
            tc.tile_pool(name="xtr", bufs=4) as xtr_pool,
            tc.tile_pool(name="w", bufs=2) as w_pool,
            tc.tile_pool(name="small", bufs=2) as small,
            tc.tile_pool(name="const", bufs=1) as cst,
            tc.tile_pool(name="outp", bufs=4) as outp,
            tc.tile_pool(name="dram", bufs=4, space="DRAM") as dramp,
            tc.tile_pool(name="acc", bufs=4, space="PSUM") as acc_pool,
            tc.tile_pool(name="ptr", bufs=2, space="PSUM") as ptr_pool,
            tc.tile_pool(name="pout", bufs=2, space="PSUM") as pout_pool,
        ):
            # ---- constants (hoisted out of the rep loop)
            ident_f = cst.tile([P, P], F32)
            make_identity(nc, ident_f)
            ident_h = cst.tile([P, P], F16)
            nc.scalar.copy(out=ident_h, in_=ident_f)
            ones_h = cst.tile([P, 8], F16)
            nc.vector.memset(ones_h, 1.0)
            gamma_bc = cst.tile([P, 1], F32)
            nc.gpsimd.dma_start(out=gamma_bc, in_=g[:].partition_broadcast(P))
            # preload the Exp table so the first real exp isn't stalled
            warm = cst.tile([P, 1], F32)
            nc.scalar.activation(out=warm, in_=gamma_bc, func=EXP)

            xr = x.rearrange("(t p) n -> p t n", p=P)
            yr = y.rearrange("(t p) n -> p t n", p=P)

            # lag-3 DRAM chain for reps > 1 (4 buffer sets, NW-wide column
            # tiles): rep r stores to set r%4, rep r reads set (r-3)%4, so
            # rep r+1's loads can prefetch a whole pipeline segment early.
            if reps > 1:
                ybrs = [
                    [
                        dramp.tile(
                            [C, NW], F32, tag=f"ybuf{i}_{ci}",
                            name=f"ybuf{i}_{ci}",
                        ).rearrange("(t p) n -> p t n", p=P)
                        for ci in range(NCH)
                    ]
                    for i in range(4)
                ]

            # ============ phase A: load -> cast -> transpose -> mm1
            def phase_a(rep):
                in_chunks = None if rep < 3 else ybrs[(rep - 3) % 4]

                Xh = xh_pool.tile([P, CB, HW], F16, tag="xh")
                pe_tiles = [
                    acc_pool.tile([P, C], F32, tag="acc", name=f"pe_{_jb}")
                    for _jb in range(CB)
                ]

                def _mm1(k, xt_k):
                    for jb in range(CB):
                        nc.tensor.matmul(
                            pe_tiles[jb][:, RS[jb]:],
                            lhsT=xt_k[:, jb * P:(jb + 1) * P],
                            rhs=xt_k[:, RS[jb]:],
                            start=(k == 0),
                            stop=(k == KB - 1),
                        )

                pend = []
                for ci in range(NCH):
                    pos = ci * NW
                    nsl = slice(pos, pos + NW)
                    src = xr[:, :, nsl] if in_chunks is None else in_chunks[ci]
                    Xs = stage.tile([P, CB, NW], F32, tag="xs")
                    nc.sync.dma_start(out=Xs, in_=src)
                    nc.scalar.copy(out=Xh[:, :, nsl], in_=Xs)
                    for kk in range(NW // P):
                        k = pos // P + kk
                        pxt = ptr_pool.tile([P, C], F16, tag="ptr", name="pxt")
                        for t in range(CB):
                            nc.tensor.transpose(
                                pxt[:, t * P:(t + 1) * P],
                                Xh[:, t, k * P:(k + 1) * P],
                                ident_h,
                            )
                        xt_k = xtr_pool.tile([P, C], F16, tag="xt")
                        nc.vector.tensor_copy(out=xt_k, in_=pxt)
                        # software-pipelined emission: this k's matmuls are
                        # issued one step later so the PE transposes the next
                        # block instead of stalling on the PSUM->SBUF copy
                        pend.append((k, xt_k))
                        if len(pend) > 2:
                            _mm1(*pend.pop(0))

                while pend:
                    _mm1(*pend.pop(0))
                return Xh, pe_tiles

            # ============ phase B1 head: symmetrize + row-min
            def phase_b1_head(pe_tiles):
                rowmin = small.tile([P, CB], F32, tag="rowmin")
                blk = small.tile([P, len(SYM), P], F32, tag="blk")
                c_sb = small.tile([P, C], F32, tag="csb")
                W = w_pool.tile([P, CB, C], F16, tag="w")
                Wt = w_pool.tile([P, CB, C], F16, tag="wt")

                def _rowmin(jb):
                    nc.vector.tensor_reduce(
                        out=rowmin[:, jb:jb + 1], in_=pe_tiles[jb],
                        axis=AX.X, op=ALU.min,
                    )

                # batched upper-block copies (sources are disjoint from every
                # transpose target, so all copies can be issued up front)
                n6 = 0
                for (sjb, scb, nb) in SYMCP:
                    nc.scalar.copy(
                        out=blk[:, n6:n6 + nb, :],
                        in_=pe_tiles[sjb][:, scb * P:(scb + nb) * P],
                    )
                    n6 += nb
                _rowmin(0)
                # blk layout from SYMCP: index n maps to (bi, bj):
                #   n 0..2 -> (1,0),(2,0),(3,0); n 3..4 -> (2,1),(3,1); n5 -> (3,2)
                blk_ix = {(1, 0): 0, (2, 0): 1, (3, 0): 2,
                          (2, 1): 3, (3, 1): 4, (3, 2): 5}
                for (bi, bj) in SYM:
                    nc.tensor.transpose(
                        pe_tiles[bi][:, bj * P:(bj + 1) * P],
                        blk[:, blk_ix[(bi, bj)], :], ident_f
                    )
                    if (bi, bj) == (1, 0):
                        _rowmin(1)
                    elif (bi, bj) == (2, 1):
                        _rowmin(2)
                    elif (bi, bj) == (3, 2):
                        _rowmin(3)
                return pe_tiles, rowmin, c_sb, W, Wt

            # ============ phase B1 tail: c broadcast + W = exp(c - E)
            # (emitted mid-mm2 of the previous rep: the PE transposes land
            # where rowmin has long finished, and the DVE queue has slack)
            def phase_b1_tail(ctx):
                pe_tiles, rowmin, c_sb, W, Wt = ctx
                # c to free-axis layout: transpose a stride-0 broadcast of
                # each rowmin column; block t of the result holds c[t*P+q]
                # replicated down all partitions.
                c_bc = ptr_pool.tile([P, C], F32, tag="ptr", name="cbc")
                for t in range(CB):
                    nc.tensor.transpose(
                        c_bc[:, t * P:(t + 1) * P],
                        rowmin[:, t:t + 1].broadcast_to([P, P]),
                        ident_f,
                    )
                nc.vector.tensor_copy(out=c_sb, in_=c_bc)

                # W = exp(c - E)  ([j_part, i_free], fp16)
                for jb in range(CB):
                    nc.vector.tensor_tensor(
                        out=Wt[:, jb, :], in0=c_sb, in1=pe_tiles[jb],
                        op=ALU.subtract,
                    )
                    nc.scalar.activation(
                        out=W[:, jb, :], in_=Wt[:, jb, :], func=EXP
                    )
                return W

            # ============ phase B2s: S sums + invsg (tiny PE work that
            # fills the gap between mm1's end and the symmetrize copies)
            def phase_b2s(W):
                invsg = small.tile([P, CB], F32, tag="invsg")
                S_ps = ptr_pool.tile([P, CB, 8], F32, tag="ptr", name="sps")
                for ib in range(CB):
                    for jb in range(CB):
                        nc.tensor.matmul(
                            S_ps[:, ib, :],
                            lhsT=W[:, jb, ib * P:(ib + 1) * P],
                            rhs=ones_h,
                            start=(jb == 0),
                            stop=(jb == CB - 1),
                        )
                nc.vector.reciprocal(out=invsg, in_=S_ps[:, :, 0])
                for ib in range(CB):
                    nc.vector.tensor_tensor(
                        out=invsg[:, ib:ib + 1], in0=invsg[:, ib:ib + 1],
                        in1=gamma_bc, op=ALU.mult,
                    )
                return invsg

            # ============ phase B2m: mm2 + epilogue + store
            def phase_b2m(rep, Xh, W, invsg, mid_hook=None):
                out_chunks = None if rep == reps - 1 else ybrs[rep % 4]

                # out = invsg_i * (W^T @ xf) + xh, chn-outer; one store/chunk
                for ci in range(NCH):
                    pos = ci * NW
                    nsl = slice(pos, pos + NW)
                    dst = yr[:, :, nsl] if out_chunks is None else out_chunks[ci]
                    osb = outp.tile([P, CB, NW], F32, tag="osb")
                    for ib in range(CB):
                        po_t = pout_pool.tile([P, NW], F32, tag="po", name="po")
                        for jb in range(CB):
                            nc.tensor.matmul(
                                po_t,
                                lhsT=W[:, jb, ib * P:(ib + 1) * P],
                                rhs=Xh[:, jb, nsl],
                                start=(jb == 0),
                                stop=(jb == CB - 1),
                            )
                        nc.vector.scalar_tensor_tensor(
                            out=osb[:, ib, :],
                            in0=po_t,
                            scalar=invsg[:, ib:ib + 1],
                            in1=Xh[:, ib, nsl],
                            op0=ALU.mult,
                            op1=ALU.add,
                        )
                    nc.scalar.dma_start(out=dst, in_=osb)
                    if ci == 3 and mid_hook is not None:
                        mid_hook()

            # ---- skewed emission: A(r), B2s(r-1), B1h(r), B2m(r-1) with
            # B1-tail(r) emitted mid-mm2 where the DVE queue has slack
            prev = None
            for _rep in range(reps):
                Xh_r, pe_r = phase_a(_rep)
                if prev is not None:
                    inv_p = phase_b2s(prev[1])
                ctx_r = phase_b1_head(pe_r)
                if prev is not None:
                    phase_b2m(
                        _rep - 1, prev[0], prev[1], inv_p,
                        mid_hook=lambda c=ctx_r: phase_b1_tail(c),
                    )
                else:
                    phase_b1_tail(ctx_r)
                prev = (Xh_r, ctx_r[3])
            inv_l = phase_b2s(prev[1])
            phase_b2m(reps - 1, prev[0], prev[1], inv_l)

    nc.compile()
    return nc


_NC_CACHE = None


def _get_nc():
    global _NC_CACHE
    if _NC_CACHE is None:
        _NC_CACHE = build_nc()
    return _NC_CACHE


def kernel(x, gamma):
    from concourse.bass_utils import run_bass_kernel_spmd

    x = np.ascontiguousarray(np.asarray(x, dtype=np.float32))
    B = x.shape[0]
    assert x.shape == (8, C, 64, 64), x.shape
    xf = x.reshape(B, C, HW)
    gamma = np.ascontiguousarray(np.asarray(gamma, dtype=np.float32)).reshape(1)

    nc = _get_nc()
    in_maps = [{"x": xf[b], "gamma": gamma} for b in range(B)]
    res = run_bass_kernel_spmd(nc, in_maps, core_ids=list(range(B)))
    out = np.stack([res.results[b]["y"] for b in range(B)], axis=0)
    return out.reshape(B, C, 64, 64).astype(np.float32)


# revision 22
# speedup vs baseline: 7.0848x; 1.9567x over previous
"""Trainium2 Bass kernel for nn_CAM (DANet channel-attention module).

Per batch element b (one per NeuronCore, 8 cores data-parallel over B=8):
    xf = x[b].reshape(C, H*W)                       # [512, 4096]
    E = xf @ xf.T                                   # [512, 512] (symmetric)
    att = softmax(max_j(E) - E, axis=-1)            # inverted softmax
    out = gamma * (att @ xf) + x[b]

Kernel math (identical in exact arithmetic to the reference):
    c[i]     = min_j E[i, j]          (= column min by symmetry)
    W[j, i]  = exp(c[i] - E[j, i])    (numerator of att[i, j]; exponent <= 0)
    S[i]     = sum_j W[j, i]
    W2[j, i] = (gamma / S[i]) * W[j, i] + I[j, i]
    out[i]   = sum_j W2[j, i] * xf[j, :]   (= gamma*att@xf + x, residual folded)

Design notes (v10 — engine assignment driven by real-HW op costs):
  - fp16 matmul path: full PE rate for matmuls AND transposes; PSUM
    accumulation fp32; the residual rides the fp16 copy of x through the
    identity diagonal of W2 (adds ~5e-4 rel err, far under the 2e-2 gate).
  - per-rep work split A (load->cast->transpose->mm1), B1 (symmetrize,
    rowmin), B2 (mm2 + store); emission is SKEWED — A(r), B1(r), B2(r-1) —
    so the PE rolls from rep r's mm1 straight into rep r-1's mm2. The
    softmax chain (c broadcast, exp, S, invsg, W2) is emitted in hooks
    between mm2 chunks of the previous rep, where every engine has slack.
  - the mm2 epilogue is a PLAIN PSUM->SBUF copy on the ACT engine: the
    TRN2 DVE pays a silicon-errata penalty plus a pipeline DRAIN per op,
    so the old invsg-scale-plus-residual-add epilogue on DVE was the
    hardware bottleneck; folding scale+residual into W2 removes it.
  - no full-size fp32 x in SBUF: loads staged in 512-col chunks, cast
    fp32->fp16 on ACT into a triple-buffered fp16 Xh.
  - loads exclusively on the SP queue; stores + epilogue copies on ACT;
    one 1MB store per chunk.
  - reps chain through DRAM with lag 3 (rep r reads rep r-3's output, 4
    buffer sets) so loads prefetch a whole pipeline segment early; the
    final rep writes the real fp32 output.

reps > 1 unrolls the whole computation serially inside one NEFF with a
true data chain between reps (used by test.py to measure steady-state
per-iteration device time).
"""

import numpy as np

import concourse.bass as bass
import concourse.mybir as mybir
import concourse.tile as tile
from concourse import bacc
from concourse.masks import make_identity

P = 128          # partitions
C = 512          # channels
HW = 4096        # spatial (64*64)
CB = C // P      # 4 channel blocks
KB = HW // P     # 32 spatial blocks
NW = 512         # matmul free-dim chunk
NCH = HW // NW   # 8 n-chunks

F32 = mybir.dt.float32
F16 = mybir.dt.float16
EXP = mybir.ActivationFunctionType.Exp
ALU = mybir.AluOpType
AX = mybir.AxisListType

# symmetry: matmuls compute only blocks (jb, ib) with ib >= jb
RS = (0, P, 2 * P, 3 * P)
SYM = ((1, 0), (2, 0), (2, 1), (3, 0), (3, 1), (3, 2))
# batched copies of the upper-triangle blocks feeding the SYM transposes:
# (src_jb, src_col_block_start, n_blocks)
SYMCP = ((0, 1, 3), (1, 2, 2), (2, 3, 1))
# blk layout from SYMCP: index n maps to (bi, bj)
BLK_IX = {(1, 0): 0, (2, 0): 1, (3, 0): 2, (2, 1): 3, (3, 1): 4, (3, 2): 5}


def build_nc(reps: int = 1):
    nc = bacc.Bacc("TRN2", target_bir_lowering=False)
    x = nc.dram_tensor("x", [C, HW], F32, kind="ExternalInput")
    g = nc.dram_tensor("gamma", [1], F32, kind="ExternalInput")
    y = nc.dram_tensor("y", [C, HW], F32, kind="ExternalOutput")

    with tile.TileContext(nc) as tc:
        with (
            tc.tile_pool(name="stage", bufs=5) as stage,
            tc.tile_pool(name="xh", bufs=3) as xh_pool,
            tc.tile_pool(name="xtr", bufs=4) as xtr_pool,
            tc.tile_pool(name="w", bufs=2) as w_pool,
            tc.tile_pool(name="small", bufs=2) as small,
            tc.tile_pool(name="const", bufs=1) as cst,
            tc.tile_pool(name="outp", bufs=4) as outp,
            tc.tile_pool(name="dram", bufs=4, space="DRAM") as dramp,
            tc.tile_pool(name="acc", bufs=4, space="PSUM") as acc_pool,
            tc.tile_pool(name="ptr", bufs=2, space="PSUM") as ptr_pool,
            tc.tile_pool(name="pout", bufs=2, space="PSUM") as pout_pool,
        ):
            # ---- constants (hoisted out of the rep loop)
            ident_f = cst.tile([P, P], F32)
            make_identity(nc, ident_f)
            ident_h = cst.tile([P, P], F16)
            nc.scalar.copy(out=ident_h, in_=ident_f)
            ones_h = cst.tile([P, 8], F16)
            nc.vector.memset(ones_h, 1.0)
            gamma_bc = cst.tile([P, 1], F32)
            nc.gpsimd.dma_start(out=gamma_bc, in_=g[:].partition_broadcast(P))
            # preload the Exp table so the first real exp isn't stalled
            warm = cst.tile([P, 1], F32)
            nc.scalar.activation(out=warm, in_=gamma_bc, func=EXP)

            xr = x.rearrange("(t p) n -> p t n", p=P)
            yr = y.rearrange("(t p) n -> p t n", p=P)

            # lag-3 DRAM chain for reps > 1 (4 buffer sets, NW-wide column
            # tiles): rep r stores to set r%4, rep r reads set (r-3)%4, so
            # rep r+1's loads can prefetch a whole pipeline segment early.
            if reps > 1:
                ybrs = [
                    [
                        dramp.tile(
                            [C, NW], F32, tag=f"ybuf{i}_{ci}",
                            name=f"ybuf{i}_{ci}",
                        ).rearrange("(t p) n -> p t n", p=P)
                        for ci in range(NCH)
                    ]
                    for i in range(4)
                ]

            # ============ phase A: load -> cast -> transpose -> mm1
            def phase_a(rep):
                in_chunks = None if rep < 3 else ybrs[(rep - 3) % 4]

                Xh = xh_pool.tile([P, CB, HW], F16, tag="xh")
                pe_tiles = [
                    acc_pool.tile([P, C], F32, tag="acc", name=f"pe_{_jb}")
                    for _jb in range(CB)
                ]

                def _mm1(k, xt_k):
                    for jb in range(CB):
                        nc.tensor.matmul(
                            pe_tiles[jb][:, RS[jb]:],
                            lhsT=xt_k[:, jb * P:(jb + 1) * P],
                            rhs=xt_k[:, RS[jb]:],
                            start=(k == 0),
                            stop=(k == KB - 1),
                        )

                pend = []
                for ci in range(NCH):
                    pos = ci * NW
                    nsl = slice(pos, pos + NW)
                    src = xr[:, :, nsl] if in_chunks is None else in_chunks[ci]
                    Xs = stage.tile([P, CB, NW], F32, tag="xs")
                    nc.sync.dma_start(out=Xs, in_=src)
                    nc.scalar.copy(out=Xh[:, :, nsl], in_=Xs)
                    for kk in range(NW // P):
                        k = pos // P + kk
                        pxt = ptr_pool.tile([P, C], F16, tag="ptr", name="pxt")
                        for t in range(CB):
                            nc.tensor.transpose(
                                pxt[:, t * P:(t + 1) * P],
                                Xh[:, t, k * P:(k + 1) * P],
                                ident_h,
                            )
                        xt_k = xtr_pool.tile([P, C], F16, tag="xt")
                        nc.vector.tensor_copy(out=xt_k, in_=pxt)
                        # software-pipelined emission: this k's matmuls are
                        # issued one step later so the PE transposes the next
                        # block instead of stalling on the PSUM->SBUF copy
                        pend.append((k, xt_k))
                        if len(pend) > 2:
                            _mm1(*pend.pop(0))

                while pend:
                    _mm1(*pend.pop(0))
                return Xh, pe_tiles

            # ============ phase B1 head: symmetrize + row-min
            def phase_b1_head(pe_tiles):
                rowmin = small.tile([P, CB], F32, tag="rowmin")
                blk = small.tile([P, len(SYM), P], F32, tag="blk")
                c_sb = small.tile([P, C], F32, tag="csb")
                invsg = small.tile([P, CB], F32, tag="invsg")
                ibc_sb = small.tile([P, C], F16, tag="ibc")
                W = w_pool.tile([P, CB, C], F16, tag="w")
                Wt = w_pool.tile([P, CB, C], F16, tag="wt")

                def _rowmin(jb):
                    nc.vector.tensor_reduce(
                        out=rowmin[:, jb:jb + 1], in_=pe_tiles[jb],
                        axis=AX.X, op=ALU.min,
                    )

                # batched upper-block copies (sources are disjoint from every
                # transpose target, so all copies can be issued up front)
                n6 = 0
                for (sjb, scb, nb) in SYMCP:
                    nc.scalar.copy(
                        out=blk[:, n6:n6 + nb, :],
                        in_=pe_tiles[sjb][:, scb * P:(scb + nb) * P],
                    )
                    n6 += nb
                _rowmin(0)
                for (bi, bj) in SYM:
                    nc.tensor.transpose(
                        pe_tiles[bi][:, bj * P:(bj + 1) * P],
                        blk[:, BLK_IX[(bi, bj)], :], ident_f
                    )
                    if (bi, bj) == (1, 0):
                        _rowmin(1)
                    elif (bi, bj) == (2, 1):
                        _rowmin(2)
                    elif (bi, bj) == (3, 2):
                        _rowmin(3)
                return pe_tiles, rowmin, c_sb, invsg, ibc_sb, W, Wt

            # ---- softmax-chain pieces, emitted between mm2 chunks of the
            # previous rep (each hook lands where its inputs are long ready
            # and steals at most ~1us from any engine queue)

            def hook_c(ctx):
                # c to free-axis layout: transpose a stride-0 broadcast of
                # each rowmin column; block t of the result holds c[t*P+q]
                # replicated down all partitions.
                pe_tiles, rowmin, c_sb = ctx[0], ctx[1], ctx[2]
                c_bc = ptr_pool.tile([P, C], F32, tag="ptr", name="cbc")
                for t in range(CB):
                    nc.tensor.transpose(
                        c_bc[:, t * P:(t + 1) * P],
                        rowmin[:, t:t + 1].broadcast_to([P, P]),
                        ident_f,
                    )
                nc.vector.tensor_copy(out=c_sb, in_=c_bc)

            def hook_w(ctx, jbs):
                # W = exp(c - E)  ([j_part, i_free], fp16)
                pe_tiles, c_sb, W, Wt = ctx[0], ctx[2], ctx[5], ctx[6]
                for jb in jbs:
                    nc.vector.tensor_tensor(
                        out=Wt[:, jb, :], in0=c_sb, in1=pe_tiles[jb],
                        op=ALU.subtract,
                    )
                    nc.scalar.activation(
                        out=W[:, jb, :], in_=Wt[:, jb, :], func=EXP
                    )

            def hook_s(ctx):
                # S_i = sum_j W[j, i]; invsg = gamma / S
                invsg, W = ctx[3], ctx[5]
                S_ps = ptr_pool.tile([P, CB, 8], F32, tag="ptr", name="sps")
                for ib in range(CB):
                    for jb in range(CB):
                        nc.tensor.matmul(
                            S_ps[:, ib, :],
                            lhsT=W[:, jb, ib * P:(ib + 1) * P],
                            rhs=ones_h,
                            start=(jb == 0),
                            stop=(jb == CB - 1),
                        )
                nc.vector.reciprocal(out=invsg, in_=S_ps[:, :, 0])
                for ib in range(CB):
                    nc.vector.tensor_tensor(
                        out=invsg[:, ib:ib + 1], in0=invsg[:, ib:ib + 1],
                        in1=gamma_bc, op=ALU.mult,
                    )

            def hook_w2(ctx):
                # W2 = invsg_i * W + I, in place over Wt: folds the softmax
                # normalization AND the residual into the mm2 weights, so
                # the mm2 epilogue is a plain PSUM->SBUF copy on ACT.
                invsg, ibc_sb, W, Wt = ctx[3], ctx[4], ctx[5], ctx[6]
                ibc = ptr_pool.tile([P, C], F32, tag="ptr", name="ibc")
                for t in range(CB):
                    nc.tensor.transpose(
                        ibc[:, t * P:(t + 1) * P],
                        invsg[:, t:t + 1].broadcast_to([P, P]),
                        ident_f,
                    )
                nc.vector.tensor_copy(out=ibc_sb, in_=ibc)
                for jb in range(CB):
                    nc.vector.tensor_tensor(
                        out=Wt[:, jb, :], in0=W[:, jb, :], in1=ibc_sb,
                        op=ALU.mult,
                    )
                    nc.vector.tensor_tensor(
                        out=Wt[:, jb, jb * P:(jb + 1) * P],
                        in0=Wt[:, jb, jb * P:(jb + 1) * P],
                        in1=ident_h, op=ALU.add,
                    )

            # ============ phase B2: mm2 (weights W2) + ACT epilogue + store
            def phase_b2(rep, Xh, W2, hooks):
                out_chunks = None if rep == reps - 1 else ybrs[rep % 4]
                for ci in range(NCH):
                    pos = ci * NW
                    nsl = slice(pos, pos + NW)
                    dst = yr[:, :, nsl] if out_chunks is None else out_chunks[ci]
                    osb = outp.tile([P, CB, NW], F32, tag="osb")
                    for ib in range(CB):
                        po_t = pout_pool.tile([P, NW], F32, tag="po", name="po")
                        for jb in range(CB):
                            nc.tensor.matmul(
                                po_t,
                                lhsT=W2[:, jb, ib * P:(ib + 1) * P],
                                rhs=Xh[:, jb, nsl],
                                start=(jb == 0),
                                stop=(jb == CB - 1),
                            )
                        nc.scalar.copy(out=osb[:, ib, :], in_=po_t)
                    nc.scalar.dma_start(out=dst, in_=osb)
                    if ci in hooks:
                        hooks[ci]()

            # ---- skewed emission: A(r), B1h(r), B2(r-1) with rep r's
            # softmax chain emitted via hooks between rep r-1's mm2 chunks
            def chain_hooks(ctx):
                return {
                    1: lambda: hook_c(ctx),
                    2: lambda: hook_w(ctx, (0, 1)),
                    3: lambda: hook_w(ctx, (2, 3)),
                    4: lambda: hook_s(ctx),
                    5: lambda: hook_w2(ctx),
                }

            prev = None
            for _rep in range(reps):
                Xh_r, pe_r = phase_a(_rep)
                ctx_r = phase_b1_head(pe_r)
                if prev is not None:
                    phase_b2(_rep - 1, prev[0], prev[1], chain_hooks(ctx_r))
                else:
                    hook_c(ctx_r)
                    hook_w(ctx_r, (0, 1, 2, 3))
                    hook_s(ctx_r)
                    hook_w2(ctx_r)
                prev = (Xh_r, ctx_r[6])
            phase_b2(reps - 1, prev[0], prev[1], {})

    nc.compile()
    return nc


_NC_CACHE = None


def _get_nc():
    global _NC_CACHE
    if _NC_CACHE is None:
        _NC_CACHE = build_nc()
    return _NC_CACHE


def kernel(x, gamma):
    from concourse.bass_utils import run_bass_kernel_spmd

    x = np.ascontiguousarray(np.asarray(x, dtype=np.float32))
    B = x.shape[0]
    assert x.shape == (8, C, 64, 64), x.shape
    xf = x.reshape(B, C, HW)
    gamma = np.ascontiguousarray(np.asarray(gamma, dtype=np.float32)).reshape(1)

    nc = _get_nc()
    in_maps = [{"x": xf[b], "gamma": gamma} for b in range(B)]
    res = run_bass_kernel_spmd(nc, in_maps, core_ids=list(range(B)))
    out = np.stack([res.results[b]["y"] for b in range(B)], axis=0)
    return out.reshape(B, C, 64, 64).astype(np.float32)
